# revision 18
# baseline (speedup 1.0000x reference)
"""Trainium2 Bass kernel for nn_BlockAttnRes.

Reference computation (B=4, N=8, S=4096, D=1024):
    partial   = partial_block + current                      [B,S,D]
    summaries = rmsnorm(block_outputs[:, :, -1, :]) * rms_w  [B,N,D]
    query     = partial[:, -1, :] @ res_proj_w.T             [B,D]
    scores    = einsum("bd,bnd->bn", query, summaries)/sqrt(D)
    weights   = softmax(scores, axis=-1)                     [B,N]
    attended  = einsum("bn,bnsd->bsd", weights, block_outputs)
    returns (partial + attended, partial)

Sharding: 8 cores, core c -> (b = c//2, s-half = c%2). Each core gets its
batch's S/2 slice of current/partial_block/block_outputs plus the (tiny)
last-token slices + replicated weights, computes its own softmax weights
(no cross-core communication), and produces its S/2 slice of both outputs.

The kernel is DMA-bound (360 GB/s per-core shared across loads+stores), so
bytes are minimized with mixed precision (harness gate is rel_err < 2e-2):
  - block_outputs: the blocks are relabel-invariant (permuting n in both
    bo and bo[:, :, -1] leaves the outputs unchanged), so the host ranks
    blocks per batch by an approximate score and streams the top NBF
    softmax-weight blocks in bf16 and the rest in fp8-e4m3. fp8's ~3%
    relative error then only carries the small softmax mass.
  - current/partial_block/outputs in bf16 (~0.2%/elem)
  - res_proj_w in bf16; last-token score-path inputs stay f32
Per-core traffic: 8+12 MiB bo + 4+4 cur/pb + 2 W + 8 outs = 38 MiB
-> ~111us floor, vs 96 MiB (274us) for the all-f32 version.

Engine budget per tile iteration (FREE=2048, NT=8, ~13.8us DMA/iter):
  sync ring : all loads (W chunks + score-path inputs strictly before
              main-loop tiles; per-iteration: ct, pt, blocks 0..7)
  scalar ring: the two stores
  PE (~7.7us): tree = ct + sum_n w[n]*bo[n] in PSUM via scaled-identity
              matmuls (lhsT = w*I in bf16, rhs = bf16/fp8 tiles, both at
              1 cycle/row; fp32 would be 4x slower)
  ACT (~3us): PSUM->SBUF copy of the tree (f32 -> bf16) + store issues
  DVE (~1.3us): partial = ct + pt via stt (2x mode)
  GpSimd    : idle (its software bf16 add measured ~9us/iter - avoid)

Known hazards baked into the structure (each cost 10-60us when violated):
  - SBUF address reuse between pools puts anti-deps on main-loop tiles;
    the first bo loads then head-of-line-block the sync ring.
  - A tile-pool slot wait on a load stalls every later load on its ring.
  - int8/1-byte dtypes get no DVE 2x mode on HW (measured 2.7us/op);
    keep the bulk path off the DVE.
  - An ACT table switch (Sqrt/Exp) costs ~1.3us; preload Exp after Sqrt.
"""

from contextlib import ExitStack

import numpy as np
import ml_dtypes

import concourse.bacc as bacc
import concourse.bass as bass
import concourse.mybir as mybir
import concourse.tile as tile
from concourse import masks
from concourse.bass_utils import run_bass_kernel_spmd

F32 = mybir.dt.float32
BF16 = mybir.dt.bfloat16
FP8 = mybir.dt.float8e4
FP32_EPS = float(np.finfo(np.float32).eps)

B, N, S, D = 4, 8, 4096, 1024
NCORES = 8
S_SH = S // 2               # 2048 sequence rows per core
P = 128                     # SBUF partitions
TWO = 2                     # s-rows packed per partition (contiguous in DRAM)
FREE = TWO * D              # 2048 elems per partition row
NT = S_SH // (P * TWO)      # 8 tiles per core
NCH = FREE // 512           # psum banks per tree tile
INV_SQRT_D = 1.0 / 32.0     # 1/sqrt(1024)
KC = D // P                 # 8 chunks of 128
NBF = 1                     # top-weight blocks streamed in bf16; rest fp8

NPBF = np.dtype(ml_dtypes.bfloat16)
NPF8 = np.dtype(ml_dtypes.float8_e4m3)


def _build_score_path(nc, tc, small, psum, wpool, persist,
                      bol, curl, pbl, w, rw):
    """Emit the tiny per-core softmax-weight computation.

    Returns (id_pe, idw): bf16 identity and per-slot w[n]*I identities
    (persist pool) for the PE accumulation.
    """
    bolt = small.tile([N, D], F32)
    nc.sync.dma_start(out=bolt[:], in_=bol.ap())
    rwt = small.tile([1, D], F32)
    nc.sync.dma_start(out=rwt[:], in_=rw.ap())
    pl = small.tile([1, D], F32)
    nc.sync.dma_start(out=pl[:], in_=curl.ap())
    pbt = small.tile([1, D], F32)
    nc.sync.dma_start(out=pbt[:], in_=pbl.ap())

    # bn path: rstd = 1/sqrt(mean(bol^2) + eps) : [N, 1]
    x2 = small.tile([N, D], F32, tag="xu")
    nc.vector.tensor_mul(out=x2[:], in0=bolt[:], in1=bolt[:])
    nsub = D // nc.vector.BN_STATS_FMAX  # 2 subgroups of 512
    stats = small.tile([N, nsub, nc.vector.BN_STATS_DIM], F32)
    x2r = x2[:].rearrange("p (s f) -> p s f", s=nsub)
    for i in range(nsub):
        nc.vector.bn_stats(out=stats[:, i, :], in_=x2r[:, i, :])
    mv = small.tile([N, nc.vector.BN_AGGR_DIM], F32)
    nc.vector.bn_aggr(out=mv[:], in_=stats[:])
    eps_t = small.tile([N, 1], F32)
    nc.vector.memset(eps_t[:], FP32_EPS)
    rstd = small.tile([N, 1], F32)
    nc.scalar.activation(
        out=rstd[:], in_=mv[:, 0:1],
        func=mybir.ActivationFunctionType.Sqrt, bias=eps_t[:], scale=1.0,
    )
    nc.vector.reciprocal(out=rstd[:], in_=rstd[:])
    # Preload the Exp activation table now (after the Sqrt, which displaces
    # it) so the softmax Exp hits a warm table.
    dummy = small.tile([1, 1], F32)
    nc.vector.memset(dummy[:], 0.0)
    nc.scalar.activation(out=dummy[:], in_=dummy[:],
                         func=mybir.ActivationFunctionType.Exp)

    # pl = (partial_block + current) last token : [1, D]
    nc.vector.tensor_add(out=pl[:], in0=pl[:], in1=pbt[:])

    # --- transposes (PE): bolT/rwT/plT per 128-chunk; sT folds rms_w and
    # is emitted in bf16 (lhsT of the u-matmul must match W's bf16) ---
    ident = small.tile([P, P], F32)
    masks.make_identity(nc, ident[:])
    sT = small.tile([P, KC, N], BF16)
    rwT = small.tile([P, KC], F32)
    plT = small.tile([P, KC], F32)
    for k in range(KC):
        ps_s = psum.tile([P, N], F32, tag="trs", bufs=1)
        nc.tensor.transpose(ps_s[:], bolt[:, k * P:(k + 1) * P], ident[:N, :N])
        ps_r = psum.tile([P, 1], F32, tag="trp", bufs=1)
        nc.tensor.transpose(ps_r[:], rwt[:, k * P:(k + 1) * P], ident[:1, :1])
        nc.vector.tensor_copy(out=rwT[:, k:k + 1], in_=ps_r[:])
        nc.vector.tensor_scalar_mul(out=sT[:, k, :], in0=ps_s[:],
                                    scalar1=rwT[:, k:k + 1])
        ps_p = psum.tile([P, 1], F32, tag="trq", bufs=1)
        nc.tensor.transpose(ps_p[:], pl[:, k * P:(k + 1) * P], ident[:1, :1])
        nc.vector.tensor_copy(out=plT[:, k:k + 1], in_=ps_p[:])

    # --- u[n, di] = sum_do s[n, do] * W[do, di]: lhsT = sT_j, rhs = W rows
    # (one bf16 DMA), accumulate over do-chunks in PSUM ---
    w_r = w.ap().rearrange("(j p) d -> p j d", p=P)
    HF = nc.tensor.MAX_MOVING_FREE_DIM_SIZE  # 512
    wal = wpool.tile([P, KC, D], BF16)
    nc.sync.dma_start(out=wal[:], in_=w_r)
    u_ps = [psum.tile([N, HF], F32, tag=f"ups{h}", bufs=1, name=f"u_ps{h}")
            for h in range(2)]
    for j in range(KC):
        for h in range(2):
            nc.tensor.matmul(
                u_ps[h][:], lhsT=sT[:, j, :],
                rhs=wal[:, j, h * HF:(h + 1) * HF],
                start=(j == 0), stop=(j == KC - 1),
            )
    # PSUM->SBUF copy of u, folding in the rstd row scale (keep f32)
    u_sb = small.tile([N, D], F32, tag="xu")
    for h in range(2):
        nc.vector.tensor_scalar_mul(out=u_sb[:, h * HF:(h + 1) * HF],
                                    in0=u_ps[h][:], scalar1=rstd[:])

    # --- transpose u chunks to uT[di, n] for the second contraction ---
    uT = small.tile([P, KC, N], F32)
    for k in range(KC):
        ps_u = psum.tile([P, N], F32, tag="tru", bufs=1)
        nc.tensor.transpose(ps_u[:], u_sb[:, k * P:(k + 1) * P], ident[:N, :N])
        nc.vector.tensor_copy(out=uT[:, k, :], in_=ps_u[:])

    # --- scores[n] = sum_di pl[di] * uT[di, n], then softmax ---
    sc_ps = psum.tile([1, N], F32, tag="scps", bufs=1)
    for k in range(KC):
        nc.tensor.matmul(
            sc_ps[:], lhsT=plT[:, k:k + 1], rhs=uT[:, k, :],
            start=(k == 0), stop=(k == KC - 1),
        )
    sc = small.tile([1, N], F32)
    nc.vector.tensor_scalar_mul(out=sc[:], in0=sc_ps[:], scalar1=INV_SQRT_D)
    mx = small.tile([1, 1], F32)
    nc.vector.reduce_max(out=mx[:], in_=sc[:], axis=mybir.AxisListType.X,
                         negate=True)
    ex = small.tile([1, N], F32)
    nc.scalar.activation(out=ex[:], in_=sc[:],
                         func=mybir.ActivationFunctionType.Exp,
                         bias=mx[:], scale=1.0)
    sm = small.tile([1, 1], F32)
    nc.vector.reduce_sum(out=sm[:], in_=ex[:], axis=mybir.AxisListType.X)
    rcp = small.tile([1, 1], F32)
    nc.vector.reciprocal(rcp[:], sm[:])
    wsm = small.tile([1, N], F32)
    nc.vector.tensor_scalar_mul(out=wsm[:], in0=ex[:], scalar1=rcp[:])

    # --- broadcast weights to all 128 partitions via ones-matmul ---
    ones = small.tile([1, P], F32)
    nc.vector.memset(ones[:], 1.0)
    wb_ps = psum.tile([P, N], F32, tag="wbps", bufs=1)
    nc.tensor.matmul(wb_ps[:], lhsT=ones[:], rhs=wsm[:], start=True, stop=True)
    wb = small.tile([P, N], F32)
    nc.vector.tensor_copy(out=wb[:], in_=wb_ps[:])

    # --- bf16 identities for the PE accumulation: plain I for the ct term
    # plus w[n]*I for every block slot ---
    id_pe = persist.tile([P, P], BF16)
    nc.vector.tensor_copy(out=id_pe[:], in_=ident[:])
    idw = persist.tile([P, N, P], BF16)
    for n in range(N):
        nc.scalar.mul(idw[:, n, :], ident[:], wb[:, n:n + 1])
    return id_pe, idw


def _build():
    nc = bacc.Bacc("TRN2", target_bir_lowering=False, debug=False)

    bobf = nc.dram_tensor("bobf", [NBF, S_SH, D], BF16, kind="ExternalInput")
    bof8 = nc.dram_tensor("bof8", [N - NBF, S_SH, D], FP8,
                          kind="ExternalInput")
    cur = nc.dram_tensor("cur", [S_SH, D], BF16, kind="ExternalInput")
    pb = nc.dram_tensor("pb", [S_SH, D], BF16, kind="ExternalInput")
    bol = nc.dram_tensor("bol", [N, D], F32, kind="ExternalInput")
    curl = nc.dram_tensor("curl", [1, D], F32, kind="ExternalInput")
    pbl = nc.dram_tensor("pbl", [1, D], F32, kind="ExternalInput")
    w = nc.dram_tensor("w", [D, D], BF16, kind="ExternalInput")
    rw = nc.dram_tensor("rw", [1, D], F32, kind="ExternalInput")
    out0 = nc.dram_tensor("out0", [S_SH, D], BF16, kind="ExternalOutput")
    out1 = nc.dram_tensor("out1", [S_SH, D], BF16, kind="ExternalOutput")

    with tile.TileContext(nc) as tc, ExitStack() as ctx:
        # One flat SBUF layout, everything resident simultaneously: no SBUF
        # address reuse between prologue and main loop (address reuse would
        # put anti-deps on the first bo loads, head-of-line-blocking the
        # sync-ring bo stream behind the prologue). PSUM pools ARE
        # sequential: the main-loop tree pool reuses the prologue's banks -
        # its first matmuls need idw anyway, so the anti-dep costs nothing.
        persist = ctx.enter_context(tc.tile_pool(name="persist", bufs=1))
        small = ctx.enter_context(tc.tile_pool(name="psmall", bufs=1))
        wpool = ctx.enter_context(tc.tile_pool(name="wpool", bufs=1))
        bbp = ctx.enter_context(tc.tile_pool(name="bbp", bufs=3))
        bfp = ctx.enter_context(tc.tile_pool(name="bfp", bufs=3))
        iop = ctx.enter_context(tc.tile_pool(name="iop", bufs=3))

        with tc.tile_pool(name="ppsum", bufs=1, space="PSUM") as psum:
            id_pe, idw = _build_score_path(
                nc, tc, small, psum, wpool, persist, bol, curl, pbl, w, rw)
        mpsum = ctx.enter_context(tc.tile_pool(name="mpsum", bufs=2,
                                               space="PSUM"))

        # ---- main loop: stream tiles; weighted sum entirely on PE ----
        # blocks are loaded in consolidated DMAs (one bf16, two fp8 halves)
        # so the PE sees one semaphore per group instead of one per block -
        # per-block waits were resetting the PE p-state every 4 matmuls
        bobf_r = bobf.ap().rearrange("n (t p two) d -> t p n (two d)",
                                     p=P, two=TWO)
        NF8A = (N - NBF) // 2
        NF8B = N - NBF - NF8A
        bof8A_r = bof8.ap()[:NF8A].rearrange("n (t p two) d -> t p n (two d)",
                                             p=P, two=TWO)
        bof8B_r = bof8.ap()[NF8A:].rearrange("n (t p two) d -> t p n (two d)",
                                             p=P, two=TWO)
        cur_r = cur.ap().rearrange("(t p two) d -> t p (two d)", p=P, two=TWO)
        pb_r = pb.ap().rearrange("(t p two) d -> t p (two d)", p=P, two=TWO)
        o0_r = out0.ap().rearrange("(t p two) d -> t p (two d)", p=P, two=TWO)
        o1_r = out1.ap().rearrange("(t p two) d -> t p (two d)", p=P, two=TWO)

        mult, add = mybir.AluOpType.mult, mybir.AluOpType.add
        for t in range(NT):
            # ct/pt load first: the tree's identity passes consume them
            # immediately, then blocks stream in PE consumption order
            ct = iop.tile([P, FREE], BF16, tag="ct")
            nc.sync.dma_start(out=ct[:], in_=cur_r[t])
            pt = iop.tile([P, FREE], BF16, tag="pt")
            nc.sync.dma_start(out=pt[:], in_=pb_r[t])
            bb = bbp.tile([P, NBF, FREE], BF16, tag="bb")
            nc.sync.dma_start(out=bb[:], in_=bobf_r[t])
            bfA = bfp.tile([P, NF8A, FREE], FP8, tag="bfA")
            nc.sync.dma_start(out=bfA[:], in_=bof8A_r[t])
            bfB = bfp.tile([P, NF8B, FREE], FP8, tag="bfB")
            nc.sync.dma_start(out=bfB[:], in_=bof8B_r[t])
            bts = [bb[:, n, :] for n in range(NBF)] + \
                  [bfA[:, n, :] for n in range(NF8A)] + \
                  [bfB[:, n, :] for n in range(NF8B)]
            # PE tree: psum_tree = ct + pt + sum_n w[n]*bo[n] via (w*I).T @ bo
            # matmuls. Split into two 2-bank halves so the PSUM pool recycles
            # at half-iteration granularity (the drain tail shrinks and the
            # PE stops stalling on bank availability late in the stream).
            trA = mpsum.tile([P, 2, 512], F32, tag="trA")
            trB = mpsum.tile([P, 2, 512], F32, tag="trB")
            halves = [trA, trB]

            def tmm(lhsT, rhs_tile, start, stop):
                for c in range(NCH):
                    tr = halves[c // 2]
                    nc.tensor.matmul(tr[:, c % 2, :], lhsT=lhsT,
                                     rhs=rhs_tile[:, c * 512:(c + 1) * 512],
                                     start=start, stop=stop)

            tmm(id_pe[:], ct, True, False)
            tmm(id_pe[:], pt, False, False)
            for n in range(N):
                tmm(idw[:, n, :], bts[n], False, n == N - 1)
            # partial = current + partial_block (DVE stt) -> out1
            pp = iop.tile([P, FREE], BF16, tag="pp")
            nc.vector.scalar_tensor_tensor(
                out=pp[:], in0=ct[:], scalar=1.0, in1=pt[:],
                op0=mult, op1=add,
            )
            nc.scalar.dma_start(out=o1_r[t], in_=pp[:])
            # ACT: per-half PSUM -> SBUF copies (f32 -> bf16) free banks
            # early and never queue behind the DVE, then one store
            o0t = iop.tile([P, FREE], BF16, tag="o0t")
            nc.scalar.copy(out=o0t[:, 0:1024],
                           in_=trA[:].rearrange("p a b -> p (a b)"))
            nc.scalar.copy(out=o0t[:, 1024:2048],
                           in_=trB[:].rearrange("p a b -> p (a b)"))
            nc.scalar.dma_start(out=o0_r[t], in_=o0t[:])

    nc.compile()
    return nc


_nc_cache = None


def _run(in_maps, trace=False):
    global _nc_cache
    if _nc_cache is None:
        _nc_cache = _build()
    return run_bass_kernel_spmd(_nc_cache, in_maps,
                                core_ids=list(range(NCORES)), trace=trace)


def _rank_blocks(current, block_outputs, partial_block, res_proj_w, rms_w):
    """Approximate per-batch softmax scores on host, for the bf16/fp8
    block-precision assignment only (the device recomputes weights
    exactly from the f32 last-token inputs)."""
    bol = block_outputs[:, :, -1, :]                      # [B,N,D]
    var = np.mean(bol * bol, axis=-1, keepdims=True)
    summ = bol / np.sqrt(var + FP32_EPS) * rms_w          # [B,N,D]
    pl = partial_block[:, -1, :] + current[:, -1, :]      # [B,D]
    q = pl @ res_proj_w.T                                 # [B,D]
    scores = np.einsum("bd,bnd->bn", q, summ)
    return np.argsort(-scores, axis=-1)                   # [B,N] descending


def _make_in_maps(current, block_outputs, partial_block, res_proj_w, rms_w):
    current = np.asarray(current, dtype=np.float32)
    block_outputs = np.asarray(block_outputs, dtype=np.float32)
    partial_block = np.asarray(partial_block, dtype=np.float32)
    res_proj_w = np.asarray(res_proj_w, dtype=np.float32)
    rms_w = np.asarray(rms_w, dtype=np.float32).reshape(1, D)

    order = _rank_blocks(current, block_outputs, partial_block,
                         res_proj_w, rms_w)
    cur_b = current.astype(NPBF)
    pb_b = partial_block.astype(NPBF)
    w_b = np.ascontiguousarray(res_proj_w.astype(NPBF))

    in_maps = []
    for c in range(NCORES):
        b, h = divmod(c, 2)
        s0 = h * S_SH
        top, rest = order[b, :NBF], order[b, NBF:]
        in_maps.append({
            "bobf": np.ascontiguousarray(
                block_outputs[b, top, s0:s0 + S_SH, :]).astype(NPBF),
            "bof8": np.ascontiguousarray(
                block_outputs[b, rest, s0:s0 + S_SH, :]).astype(NPF8),
            "cur": np.ascontiguousarray(cur_b[b, s0:s0 + S_SH, :]),
            "pb": np.ascontiguousarray(pb_b[b, s0:s0 + S_SH, :]),
            "bol": np.ascontiguousarray(block_outputs[b, order[b], -1, :]),
            "curl": np.ascontiguousarray(current[b, -1:, :]),
            "pbl": np.ascontiguousarray(partial_block[b, -1:, :]),
            "w": w_b,
            "rw": np.ascontiguousarray(rms_w),
        })
    return in_maps


def _gather(results):
    out0 = np.empty((B, S, D), np.float32)
    out1 = np.empty((B, S, D), np.float32)
    for c in range(NCORES):
        b, h = divmod(c, 2)
        s0 = h * S_SH
        out0[b, s0:s0 + S_SH, :] = results[c]["out0"].astype(np.float32)
        out1[b, s0:s0 + S_SH, :] = results[c]["out1"].astype(np.float32)
    return out0, out1


def kernel(current, block_outputs, partial_block, res_proj_w, rms_w):
    in_maps = _make_in_maps(current, block_outputs, partial_block,
                            res_proj_w, rms_w)
    res = _run(in_maps, trace=False)
    return _gather(res.results)


# revision 19
# speedup vs baseline: 1.0703x; 1.0703x over previous
"""Trainium2 Bass kernel for nn_BlockAttnRes.

Reference computation (B=4, N=8, S=4096, D=1024):
    partial   = partial_block + current                      [B,S,D]
    summaries = rmsnorm(block_outputs[:, :, -1, :]) * rms_w  [B,N,D]
    query     = partial[:, -1, :] @ res_proj_w.T             [B,D]
    scores    = einsum("bd,bnd->bn", query, summaries)/sqrt(D)
    weights   = softmax(scores, axis=-1)                     [B,N]
    attended  = einsum("bn,bnsd->bsd", weights, block_outputs)
    returns (partial + attended, partial)

Sharding: 8 cores, core c -> (b = c//2, s-half = c%2). Each core gets its
batch's S/2 slice of current/partial_block/block_outputs plus the (tiny)
last-token slices + replicated weights, computes its own softmax weights
(no cross-core communication), and produces its S/2 slice of both outputs.

The kernel is DMA-bound (360 GB/s per-core shared across loads+stores), so
bytes are minimized with mixed precision (harness gate is rel_err < 2e-2):
  - block_outputs: the blocks are relabel-invariant (permuting n in both
    bo and bo[:, :, -1] leaves the outputs unchanged), so the host ranks
    blocks per batch by an approximate score and streams the top NBF
    softmax-weight blocks in bf16 and the rest in fp8-e4m3. fp8's ~3%
    relative error then only carries the small softmax mass.
  - current/partial_block/outputs in bf16 (~0.2%/elem)
  - res_proj_w in bf16; last-token score-path inputs stay f32
Per-core traffic: 8+12 MiB bo + 4+4 cur/pb + 2 W + 8 outs = 38 MiB
-> ~111us floor, vs 96 MiB (274us) for the all-f32 version.

Engine budget per tile iteration (FREE=2048, NT=8, ~13.8us DMA/iter):
  sync ring : all loads (W chunks + score-path inputs strictly before
              main-loop tiles; per-iteration: ct, pt, blocks 0..7)
  scalar ring: the two stores
  PE (~7.7us): tree = ct + sum_n w[n]*bo[n] in PSUM via scaled-identity
              matmuls (lhsT = w*I in bf16, rhs = bf16/fp8 tiles, both at
              1 cycle/row; fp32 would be 4x slower)
  ACT (~3us): PSUM->SBUF copy of the tree (f32 -> bf16) + store issues
  DVE (~1.3us): partial = ct + pt via stt (2x mode)
  GpSimd    : idle (its software bf16 add measured ~9us/iter - avoid)

Known hazards baked into the structure (each cost 10-60us when violated):
  - SBUF address reuse between pools puts anti-deps on main-loop tiles;
    the first bo loads then head-of-line-block the sync ring.
  - A tile-pool slot wait on a load stalls every later load on its ring.
  - int8/1-byte dtypes get no DVE 2x mode on HW (measured 2.7us/op);
    keep the bulk path off the DVE.
  - An ACT table switch (Sqrt/Exp) costs ~1.3us; preload Exp after Sqrt.
"""

from contextlib import ExitStack

import numpy as np
import ml_dtypes

import concourse.bacc as bacc
import concourse.bass as bass
import concourse.mybir as mybir
import concourse.tile as tile
from concourse import masks
from concourse.bass_utils import run_bass_kernel_spmd

F32 = mybir.dt.float32
BF16 = mybir.dt.bfloat16
FP8 = mybir.dt.float8e4
FP32_EPS = float(np.finfo(np.float32).eps)

B, N, S, D = 4, 8, 4096, 1024
NCORES = 8
S_SH = S // 2               # 2048 sequence rows per core
P = 128                     # SBUF partitions
TWO = 2                     # s-rows packed per partition (contiguous in DRAM)
FREE = TWO * D              # 2048 elems per partition row
NT = S_SH // (P * TWO)      # 8 tiles per core
NCH = FREE // 512           # psum banks per tree tile
INV_SQRT_D = 1.0 / 32.0     # 1/sqrt(1024)
KC = D // P                 # 8 chunks of 128
NBF = 2                     # top-weight blocks streamed in bf16; rest fp8

NPBF = np.dtype(ml_dtypes.bfloat16)
NPF8 = np.dtype(ml_dtypes.float8_e4m3)


def _build_score_path(nc, tc, small, psum, wpool, persist,
                      bol, curl, pbl, w, rw):
    """Emit the tiny per-core softmax-weight computation.

    Returns (id_pe, idw): bf16 identity and per-slot w[n]*I identities
    (persist pool) for the PE accumulation.
    """
    bolt = small.tile([N, D], F32)
    nc.sync.dma_start(out=bolt[:], in_=bol.ap())
    rwt = small.tile([1, D], F32)
    nc.sync.dma_start(out=rwt[:], in_=rw.ap())
    pl = small.tile([1, D], F32)
    nc.sync.dma_start(out=pl[:], in_=curl.ap())
    pbt = small.tile([1, D], F32)
    nc.sync.dma_start(out=pbt[:], in_=pbl.ap())

    # bn path: rstd = 1/sqrt(mean(bol^2) + eps) : [N, 1]
    x2 = small.tile([N, D], F32, tag="xu")
    nc.vector.tensor_mul(out=x2[:], in0=bolt[:], in1=bolt[:])
    nsub = D // nc.vector.BN_STATS_FMAX  # 2 subgroups of 512
    stats = small.tile([N, nsub, nc.vector.BN_STATS_DIM], F32)
    x2r = x2[:].rearrange("p (s f) -> p s f", s=nsub)
    for i in range(nsub):
        nc.vector.bn_stats(out=stats[:, i, :], in_=x2r[:, i, :])
    mv = small.tile([N, nc.vector.BN_AGGR_DIM], F32)
    nc.vector.bn_aggr(out=mv[:], in_=stats[:])
    eps_t = small.tile([N, 1], F32)
    nc.vector.memset(eps_t[:], FP32_EPS)
    rstd = small.tile([N, 1], F32)
    nc.scalar.activation(
        out=rstd[:], in_=mv[:, 0:1],
        func=mybir.ActivationFunctionType.Sqrt, bias=eps_t[:], scale=1.0,
    )
    nc.vector.reciprocal(out=rstd[:], in_=rstd[:])
    # Preload the Exp activation table now (after the Sqrt, which displaces
    # it) so the softmax Exp hits a warm table.
    dummy = small.tile([1, 1], F32)
    nc.vector.memset(dummy[:], 0.0)
    nc.scalar.activation(out=dummy[:], in_=dummy[:],
                         func=mybir.ActivationFunctionType.Exp)

    # pl = (partial_block + current) last token : [1, D]
    nc.vector.tensor_add(out=pl[:], in0=pl[:], in1=pbt[:])

    # --- transposes (PE): bolT/rwT/plT per 128-chunk; sT folds rms_w and
    # is emitted in bf16 (lhsT of the u-matmul must match W's bf16) ---
    ident = small.tile([P, P], F32)
    masks.make_identity(nc, ident[:])
    sT = small.tile([P, KC, N], BF16)
    rwT = small.tile([P, KC], F32)
    plT = small.tile([P, KC], F32)
    for k in range(KC):
        ps_s = psum.tile([P, N], F32, tag="trs", bufs=1)
        nc.tensor.transpose(ps_s[:], bolt[:, k * P:(k + 1) * P], ident[:N, :N])
        ps_r = psum.tile([P, 1], F32, tag="trp", bufs=1)
        nc.tensor.transpose(ps_r[:], rwt[:, k * P:(k + 1) * P], ident[:1, :1])
        nc.vector.tensor_copy(out=rwT[:, k:k + 1], in_=ps_r[:])
        nc.vector.tensor_scalar_mul(out=sT[:, k, :], in0=ps_s[:],
                                    scalar1=rwT[:, k:k + 1])
        ps_p = psum.tile([P, 1], F32, tag="trq", bufs=1)
        nc.tensor.transpose(ps_p[:], pl[:, k * P:(k + 1) * P], ident[:1, :1])
        nc.vector.tensor_copy(out=plT[:, k:k + 1], in_=ps_p[:])

    # --- u[n, di] = sum_do s[n, do] * W[do, di]: lhsT = sT_j, rhs = W rows
    # (one bf16 DMA), accumulate over do-chunks in PSUM ---
    w_r = w.ap().rearrange("(j p) d -> p j d", p=P)
    HF = nc.tensor.MAX_MOVING_FREE_DIM_SIZE  # 512
    wal = wpool.tile([P, KC, D], BF16)
    nc.sync.dma_start(out=wal[:], in_=w_r)
    u_ps = [psum.tile([N, HF], F32, tag=f"ups{h}", bufs=1, name=f"u_ps{h}")
            for h in range(2)]
    for j in range(KC):
        for h in range(2):
            nc.tensor.matmul(
                u_ps[h][:], lhsT=sT[:, j, :],
                rhs=wal[:, j, h * HF:(h + 1) * HF],
                start=(j == 0), stop=(j == KC - 1),
            )
    # PSUM->SBUF copy of u, folding in the rstd row scale (keep f32)
    u_sb = small.tile([N, D], F32, tag="xu")
    for h in range(2):
        nc.vector.tensor_scalar_mul(out=u_sb[:, h * HF:(h + 1) * HF],
                                    in0=u_ps[h][:], scalar1=rstd[:])

    # --- transpose u chunks to uT[di, n] for the second contraction ---
    uT = small.tile([P, KC, N], F32)
    for k in range(KC):
        ps_u = psum.tile([P, N], F32, tag="tru", bufs=1)
        nc.tensor.transpose(ps_u[:], u_sb[:, k * P:(k + 1) * P], ident[:N, :N])
        nc.vector.tensor_copy(out=uT[:, k, :], in_=ps_u[:])

    # --- scores[n] = sum_di pl[di] * uT[di, n], then softmax ---
    sc_ps = psum.tile([1, N], F32, tag="scps", bufs=1)
    for k in range(KC):
        nc.tensor.matmul(
            sc_ps[:], lhsT=plT[:, k:k + 1], rhs=uT[:, k, :],
            start=(k == 0), stop=(k == KC - 1),
        )
    sc = small.tile([1, N], F32)
    nc.vector.tensor_scalar_mul(out=sc[:], in0=sc_ps[:], scalar1=INV_SQRT_D)
    mx = small.tile([1, 1], F32)
    nc.vector.reduce_max(out=mx[:], in_=sc[:], axis=mybir.AxisListType.X,
                         negate=True)
    ex = small.tile([1, N], F32)
    nc.scalar.activation(out=ex[:], in_=sc[:],
                         func=mybir.ActivationFunctionType.Exp,
                         bias=mx[:], scale=1.0)
    sm = small.tile([1, 1], F32)
    nc.vector.reduce_sum(out=sm[:], in_=ex[:], axis=mybir.AxisListType.X)
    rcp = small.tile([1, 1], F32)
    nc.vector.reciprocal(rcp[:], sm[:])
    wsm = small.tile([1, N], F32)
    nc.vector.tensor_scalar_mul(out=wsm[:], in0=ex[:], scalar1=rcp[:])

    # --- broadcast weights to all 128 partitions via ones-matmul ---
    ones = small.tile([1, P], F32)
    nc.vector.memset(ones[:], 1.0)
    wb_ps = psum.tile([P, N], F32, tag="wbps", bufs=1)
    nc.tensor.matmul(wb_ps[:], lhsT=ones[:], rhs=wsm[:], start=True, stop=True)
    wb = small.tile([P, N], F32)
    nc.vector.tensor_copy(out=wb[:], in_=wb_ps[:])

    # --- bf16 identities for the PE accumulation: plain I for the ct term
    # plus w[n]*I for every block slot ---
    id_pe = persist.tile([P, P], BF16)
    nc.vector.tensor_copy(out=id_pe[:], in_=ident[:])
    idw = persist.tile([P, N, P], BF16)
    for n in range(N):
        nc.scalar.mul(idw[:, n, :], ident[:], wb[:, n:n + 1])
    return id_pe, idw


def _build():
    nc = bacc.Bacc("TRN2", target_bir_lowering=False, debug=False)

    bobf = nc.dram_tensor("bobf", [NBF, S_SH, D], BF16, kind="ExternalInput")
    bof8 = nc.dram_tensor("bof8", [N - NBF, S_SH, D], FP8,
                          kind="ExternalInput")
    cur = nc.dram_tensor("cur", [S_SH, D], BF16, kind="ExternalInput")
    pb = nc.dram_tensor("pb", [S_SH, D], BF16, kind="ExternalInput")
    bol = nc.dram_tensor("bol", [N, D], F32, kind="ExternalInput")
    curl = nc.dram_tensor("curl", [1, D], F32, kind="ExternalInput")
    pbl = nc.dram_tensor("pbl", [1, D], F32, kind="ExternalInput")
    w = nc.dram_tensor("w", [D, D], BF16, kind="ExternalInput")
    rw = nc.dram_tensor("rw", [1, D], F32, kind="ExternalInput")
    out0 = nc.dram_tensor("out0", [S_SH, D], BF16, kind="ExternalOutput")
    out1 = nc.dram_tensor("out1", [S_SH, D], BF16, kind="ExternalOutput")

    with tile.TileContext(nc) as tc, ExitStack() as ctx:
        # One flat SBUF layout, everything resident simultaneously: no SBUF
        # address reuse between prologue and main loop (address reuse would
        # put anti-deps on the first bo loads, head-of-line-blocking the
        # sync-ring bo stream behind the prologue). PSUM pools ARE
        # sequential: the main-loop tree pool reuses the prologue's banks -
        # its first matmuls need idw anyway, so the anti-dep costs nothing.
        persist = ctx.enter_context(tc.tile_pool(name="persist", bufs=1))
        small = ctx.enter_context(tc.tile_pool(name="psmall", bufs=1))
        wpool = ctx.enter_context(tc.tile_pool(name="wpool", bufs=1))
        bbp = ctx.enter_context(tc.tile_pool(name="bbp", bufs=3))
        bfp = ctx.enter_context(tc.tile_pool(name="bfp", bufs=3))
        iop = ctx.enter_context(tc.tile_pool(name="iop", bufs=3))

        with tc.tile_pool(name="ppsum", bufs=1, space="PSUM") as psum:
            id_pe, idw = _build_score_path(
                nc, tc, small, psum, wpool, persist, bol, curl, pbl, w, rw)
        mpsum = ctx.enter_context(tc.tile_pool(name="mpsum", bufs=2,
                                               space="PSUM"))

        # ---- main loop: stream tiles; weighted sum entirely on PE ----
        # blocks are loaded in consolidated DMAs (one bf16, two fp8 halves)
        # so the PE sees one semaphore per group instead of one per block -
        # per-block waits were resetting the PE p-state every 4 matmuls
        bobf_r = bobf.ap().rearrange("n (t p two) d -> t p n (two d)",
                                     p=P, two=TWO)
        NF8A = (N - NBF) // 2
        NF8B = N - NBF - NF8A
        bof8A_r = bof8.ap()[:NF8A].rearrange("n (t p two) d -> t p n (two d)",
                                             p=P, two=TWO)
        bof8B_r = bof8.ap()[NF8A:].rearrange("n (t p two) d -> t p n (two d)",
                                             p=P, two=TWO)
        cur_r = cur.ap().rearrange("(t p two) d -> t p (two d)", p=P, two=TWO)
        pb_r = pb.ap().rearrange("(t p two) d -> t p (two d)", p=P, two=TWO)
        o0_r = out0.ap().rearrange("(t p two) d -> t p (two d)", p=P, two=TWO)
        o1_r = out1.ap().rearrange("(t p two) d -> t p (two d)", p=P, two=TWO)

        mult, add = mybir.AluOpType.mult, mybir.AluOpType.add
        for t in range(NT):
            # ct/pt load first: the tree's identity passes consume them
            # immediately, then blocks stream in PE consumption order
            ct = iop.tile([P, FREE], BF16, tag="ct")
            nc.sync.dma_start(out=ct[:], in_=cur_r[t])
            pt = iop.tile([P, FREE], BF16, tag="pt")
            nc.sync.dma_start(out=pt[:], in_=pb_r[t])
            bb = bbp.tile([P, NBF, FREE], BF16, tag="bb")
            nc.sync.dma_start(out=bb[:], in_=bobf_r[t])
            bfA = bfp.tile([P, NF8A, FREE], FP8, tag="bfA")
            nc.sync.dma_start(out=bfA[:], in_=bof8A_r[t])
            bfB = bfp.tile([P, NF8B, FREE], FP8, tag="bfB")
            nc.sync.dma_start(out=bfB[:], in_=bof8B_r[t])
            bts = [bb[:, n, :] for n in range(NBF)] + \
                  [bfA[:, n, :] for n in range(NF8A)] + \
                  [bfB[:, n, :] for n in range(NF8B)]
            # PE tree: psum_tree = ct + pt + sum_n w[n]*bo[n] via (w*I).T @ bo
            # matmuls. Split into two 2-bank halves so the PSUM pool recycles
            # at half-iteration granularity (the drain tail shrinks and the
            # PE stops stalling on bank availability late in the stream).
            trA = mpsum.tile([P, 2, 512], F32, tag="trA")
            trB = mpsum.tile([P, 2, 512], F32, tag="trB")
            halves = [trA, trB]

            def tmm(lhsT, rhs_tile, start, stop):
                for c in range(NCH):
                    tr = halves[c // 2]
                    nc.tensor.matmul(tr[:, c % 2, :], lhsT=lhsT,
                                     rhs=rhs_tile[:, c * 512:(c + 1) * 512],
                                     start=start, stop=stop)

            tmm(id_pe[:], ct, True, False)
            tmm(id_pe[:], pt, False, False)
            for n in range(N):
                tmm(idw[:, n, :], bts[n], False, n == N - 1)
            # partial = current + partial_block (DVE stt) -> out1
            pp = iop.tile([P, FREE], BF16, tag="pp")
            nc.vector.scalar_tensor_tensor(
                out=pp[:], in0=ct[:], scalar=1.0, in1=pt[:],
                op0=mult, op1=add,
            )
            nc.scalar.dma_start(out=o1_r[t], in_=pp[:])
            # ACT: per-half PSUM -> SBUF copies (f32 -> bf16) free banks
            # early and never queue behind the DVE, then one store
            o0t = iop.tile([P, FREE], BF16, tag="o0t")
            nc.scalar.copy(out=o0t[:, 0:1024],
                           in_=trA[:].rearrange("p a b -> p (a b)"))
            nc.scalar.copy(out=o0t[:, 1024:2048],
                           in_=trB[:].rearrange("p a b -> p (a b)"))
            nc.scalar.dma_start(out=o0_r[t], in_=o0t[:])

    nc.compile()
    return nc


_nc_cache = None


def _run(in_maps, trace=False):
    global _nc_cache
    if _nc_cache is None:
        _nc_cache = _build()
    return run_bass_kernel_spmd(_nc_cache, in_maps,
                                core_ids=list(range(NCORES)), trace=trace)


def _rank_blocks(current, block_outputs, partial_block, res_proj_w, rms_w):
    """Approximate per-batch softmax scores on host, for the bf16/fp8
    block-precision assignment only (the device recomputes weights
    exactly from the f32 last-token inputs)."""
    bol = block_outputs[:, :, -1, :]                      # [B,N,D]
    var = np.mean(bol * bol, axis=-1, keepdims=True)
    summ = bol / np.sqrt(var + FP32_EPS) * rms_w          # [B,N,D]
    pl = partial_block[:, -1, :] + current[:, -1, :]      # [B,D]
    q = pl @ res_proj_w.T                                 # [B,D]
    scores = np.einsum("bd,bnd->bn", q, summ)
    return np.argsort(-scores, axis=-1)                   # [B,N] descending


def _make_in_maps(current, block_outputs, partial_block, res_proj_w, rms_w):
    current = np.asarray(current, dtype=np.float32)
    block_outputs = np.asarray(block_outputs, dtype=np.float32)
    partial_block = np.asarray(partial_block, dtype=np.float32)
    res_proj_w = np.asarray(res_proj_w, dtype=np.float32)
    rms_w = np.asarray(rms_w, dtype=np.float32).reshape(1, D)

    order = _rank_blocks(current, block_outputs, partial_block,
                         res_proj_w, rms_w)
    cur_b = current.astype(NPBF)
    pb_b = partial_block.astype(NPBF)
    w_b = np.ascontiguousarray(res_proj_w.astype(NPBF))

    in_maps = []
    for c in range(NCORES):
        b, h = divmod(c, 2)
        s0 = h * S_SH
        top, rest = order[b, :NBF], order[b, NBF:]
        in_maps.append({
            "bobf": np.ascontiguousarray(
                block_outputs[b, top, s0:s0 + S_SH, :]).astype(NPBF),
            "bof8": np.ascontiguousarray(
                block_outputs[b, rest, s0:s0 + S_SH, :]).astype(NPF8),
            "cur": np.ascontiguousarray(cur_b[b, s0:s0 + S_SH, :]),
            "pb": np.ascontiguousarray(pb_b[b, s0:s0 + S_SH, :]),
            "bol": np.ascontiguousarray(block_outputs[b, order[b], -1, :]),
            "curl": np.ascontiguousarray(current[b, -1:, :]),
            "pbl": np.ascontiguousarray(partial_block[b, -1:, :]),
            "w": w_b,
            "rw": np.ascontiguousarray(rms_w),
        })
    return in_maps


def _gather(results):
    out0 = np.empty((B, S, D), np.float32)
    out1 = np.empty((B, S, D), np.float32)
    for c in range(NCORES):
        b, h = divmod(c, 2)
        s0 = h * S_SH
        out0[b, s0:s0 + S_SH, :] = results[c]["out0"].astype(np.float32)
        out1[b, s0:s0 + S_SH, :] = results[c]["out1"].astype(np.float32)
    return out0, out1


def kernel(current, block_outputs, partial_block, res_proj_w, rms_w):
    in_maps = _make_in_maps(current, block_outputs, partial_block,
                            res_proj_w, rms_w)
    res = _run(in_maps, trace=False)
    return _gather(res.results)
